# revision 13
# baseline (speedup 1.0000x reference)
"""HANModel kernel for 8 Trainium2 NeuronCores — full on-device implementation.

Sharding: destination-node (news) partition, 8 x 3750 dsts per core, per the
spec hint. Projection is sharded by node and the per-node tables are
all-gathered on-device; message passing, semantic attention and the output
head all run on-device; the only cross-core traffic is the table AllGather
and a tiny score AllReduce.

Math trick that makes message passing cheap: since exp is monotonic,
  exp(leaky_relu(asrc+adst)) = max(exp(asrc)*exp(adst),
                                   exp(.2*asrc)*exp(.2*adst))
so per-edge attention weights factorize into a gathered src-side pair
(EA, EB) and a per-dst pair (ED, EF).  Edges are laid out degree-bucketed:
partition = dst row (within a 128-dst block), free dim = edge slots.  All
per-edge compute is then plain vector ops with per-partition broadcasts;
the softmax max-subtraction cancels exactly and is skipped.

Tables (per node, 512B rows, fp16): [h(128) | EA(8) | EB(8) | pad].
Dummy row NV holds zeros so padding slots contribute exactly 0.
"""
import os
import sys
import time

import numpy as np

sys.path.insert(0, "/opt/trn_rl_repo")

H, D_HEAD = 8, 16
HID = H * D_HEAD
N_NEWS, N_INTER, F_IN, C_OUT = 30000, 60000, 768, 4
NC = 8
ND = N_NEWS // NC            # 3750 dsts per core
NB = 30                      # dst blocks of 128 (3840 padded)
NDP = NB * 128               # 3840
NV = N_NEWS                  # dummy table row index (tables have NV+pad rows)
NTAB = NV + 128              # table rows padded
KC = F_IN // 128             # 6 contraction chunks

_LAST = {"ns": None, "nc": None, "in_maps": None}


# --------------------------------------------------------------------------
# host-side helpers
# --------------------------------------------------------------------------
def _a_fold(a):
    """[H, D] head vectors -> [HID, H] fold matrix (W @ A gives per-head dots)."""
    A = np.zeros((HID, H), np.float32)
    for h in range(H):
        A[h * D_HEAD:(h + 1) * D_HEAD, h] = a[h]
    return A


def _wrap_idx(flat):
    """int16 flat idx -> [128, n/16] wrapped/replicated layout for dma_gather."""
    assert flat.size % 16 == 0
    w = flat.astype(np.int16).reshape(-1, 16).T
    return np.tile(w, (8, 1))


def _prep_edges(edge, n_dst_lo):
    """Bucket one edge type for one core: returns (idx_mat [3750, degmax], deg)."""
    src = np.asarray(edge[0], np.int64)
    dst = np.asarray(edge[1], np.int64)
    m = (dst >= n_dst_lo) & (dst < n_dst_lo + ND)
    src, dstl = src[m], dst[m] - n_dst_lo
    deg = np.bincount(dstl, minlength=ND).astype(np.int64)
    order = np.argsort(dstl, kind="stable")
    ss, ds = src[order], dstl[order]
    starts = np.zeros(ND, np.int64)
    starts[1:] = np.cumsum(deg)[:-1]
    pos = np.arange(ss.size) - starts[ds]
    degmax = int(deg.max()) if deg.size else 1
    mat = np.full((ND, max(degmax, 1)), NV, np.int64)
    mat[ds, pos] = ss
    return mat, deg


def _np_full_reference(inp):
    """Host fallback (input-general, slow)."""
    x_news = inp["x_news"].astype(np.float32)
    x_inter = inp["x_inter"].astype(np.float32)

    def han(h_src, h_dst, edge, a_src, a_dst):
        asrc = (h_src.reshape(-1, H, D_HEAD) * a_src).sum(-1)
        adst = (h_dst.reshape(-1, H, D_HEAD) * a_dst).sum(-1)
        s, d = edge[0].astype(np.int64), edge[1].astype(np.int64)
        e = asrc[s] + adst[d]
        e = np.where(e > 0, e, np.float32(0.2) * e)
        mx = np.full((N_NEWS, H), -np.inf, np.float32)
        np.maximum.at(mx, d, e)
        mx = np.where(np.isfinite(mx), mx, np.float32(0))
        ex = np.exp(e - mx[d])
        den = np.zeros((N_NEWS, H), np.float32)
        np.add.at(den, d, ex)
        alpha = ex / (den[d] + np.float32(1e-16))
        out = np.zeros((N_NEWS, H, D_HEAD), np.float32)
        np.add.at(out, d, h_src[s].reshape(-1, H, D_HEAD) * alpha[:, :, None])
        return np.maximum(out.reshape(N_NEWS, HID), 0.0)

    h_news = x_news @ inp["W_news"].astype(np.float32) + inp["b_news"]
    h_inter = x_inter @ inp["W_inter"].astype(np.float32) + inp["b_inter"]
    out_nn = han(h_news, h_news, inp["edge_nn"], inp["a_src_nn"], inp["a_dst_nn"])
    out_in = han(h_inter, h_news, inp["edge_in"], inp["a_src_in"], inp["a_dst_in"])
    outs = np.stack([out_nn, out_in])
    t = np.tanh(outs @ inp["Wk"].astype(np.float32) + inp["bk"])
    score = (inp["q"].astype(np.float32) * t.mean(axis=1)).sum(-1)
    e = np.exp(score - score.max())
    beta = e / e.sum()
    fused = (beta[:, None, None] * outs).sum(0)
    elu = np.where(fused > 0, fused, np.exp(np.minimum(fused, 0.0)) - 1.0)
    return (elu @ inp["W_out"].astype(np.float32) + inp["b_out"]).astype(np.float32)


# --------------------------------------------------------------------------
# device graph
# --------------------------------------------------------------------------
def _build_graph(Dnn, Din, phases=None):
    import concourse.bacc as bacc
    import concourse.bass_isa as bass_isa
    import concourse.mybir as mybir
    import concourse.tile as tile
    from concourse import library_config

    if phases is None:
        phases = os.environ.get("KERNEL_PHASES", "pms")
    MP = "m" in phases
    SEM = "s" in phases

    f16, f32, i16 = mybir.dt.float16, mybir.dt.float32, mybir.dt.int16
    ALU = mybir.AluOpType
    ACTF = mybir.ActivationFunctionType

    snn, sin = int(np.sum(Dnn)), int(np.sum(Din))

    nc = bacc.Bacc("TRN2", num_devices=NC)
    xnT = nc.dram_tensor("xnT", [F_IN, ND], f16, kind="ExternalInput")
    xiT = nc.dram_tensor("xiT", [F_IN, ND], f16, kind="ExternalInput")
    wn_d = nc.dram_tensor("wn", [F_IN, 152], f16, kind="ExternalInput")
    wi_d = nc.dram_tensor("wi", [F_IN, 136], f16, kind="ExternalInput")
    bn_d = nc.dram_tensor("bn", [128, 152], f32, kind="ExternalInput")
    bi_d = nc.dram_tensor("bi", [128, 136], f32, kind="ExternalInput")
    inn_d = nc.dram_tensor("idx_nn", [128, 8 * snn], i16, kind="ExternalInput")
    iin_d = nc.dram_tensor("idx_in", [128, 8 * sin], i16, kind="ExternalInput")
    pix_d = nc.dram_tensor("pidx", [128, NDP // 16], i16, kind="ExternalInput")
    wk_d = nc.dram_tensor("wk", [128, 128], f16, kind="ExternalInput")
    bk_d = nc.dram_tensor("bkr", [128, 128], f32, kind="ExternalInput")
    q_d = nc.dram_tensor("qr", [128, 128], f16, kind="ExternalInput")
    wo_d = nc.dram_tensor("wo", [128, C_OUT], f16, kind="ExternalInput")
    bo_d = nc.dram_tensor("bor", [128, C_OUT], f32, kind="ExternalInput")
    id_d = nc.dram_tensor("ident", [128, 128], f16, kind="ExternalInput")
    out_d = nc.dram_tensor("out4", [NDP, C_OUT], f32, kind="ExternalOutput")

    shN = nc.dram_tensor("shN", [ND, 256], f16)
    shI = nc.dram_tensor("shI", [ND, 256], f16)
    tabN = nc.dram_tensor("tabN", [NTAB, 256], f16)
    tabI = nc.dram_tensor("tabI", [NTAB, 256], f16)
    edefT = nc.dram_tensor("edefT", [NDP, 128], f16)
    arI = nc.dram_tensor("arI", [128, 2], f32)
    arO = nc.dram_tensor("arO", [128, 2], f32)

    with tile.TileContext(nc) as tc:
        nc.gpsimd.load_library(library_config.mlp)
        with (
            tc.tile_pool(name="const", bufs=1) as cp,
            tc.tile_pool(name="acc", bufs=1) as accp,
        ):
            ident = cp.tile([128, 128], f16, tag="ident")
            nc.sync.dma_start(ident[:], id_d[:])
            wk_t = cp.tile([128, 128], f16, tag="wk")
            nc.sync.dma_start(wk_t[:], wk_d[:])
            bk_t = cp.tile([128, 128], f32, tag="bk")
            nc.sync.dma_start(bk_t[:], bk_d[:])
            q_t = cp.tile([128, 128], f16, tag="q")
            nc.sync.dma_start(q_t[:], q_d[:])
            wo_t = cp.tile([128, C_OUT], f16, tag="wo")
            nc.sync.dma_start(wo_t[:], wo_d[:])
            bo_t = cp.tile([128, C_OUT], f32, tag="bo")
            nc.sync.dma_start(bo_t[:], bo_d[:])

            outR = [accp.tile([128, NB, 128], f16, tag=f"outR{t}",
                              name=f"outR{t}") for t in range(2)]
            for t in range(2):
                nc.vector.memset(outR[t][:], 0.0)
            sacc = [accp.tile([128, 1], f32, tag=f"sacc{t}",
                              name=f"sacc{t}") for t in range(2)]
            edef_all = accp.tile([128, NB, 128], f16, tag="edef")

            # ---------------- projection ----------------
            with (
                tc.tile_pool(name="xw", bufs=1) as xp,
                tc.tile_pool(name="prow", bufs=4) as rp,
                tc.tile_pool(name="pps", bufs=3, space="PSUM") as pp,
            ):
                for (xdr, wdr, bdr, ncol, shdr, is_news) in (
                    (xnT, wn_d, bn_d, 152, shN, True),
                    (xiT, wi_d, bi_d, 136, shI, False),
                ):
                    xt = xp.tile([128, KC, ND], f16, tag="x")
                    wt = xp.tile([128, KC, ncol], f16, tag="w")
                    bt = xp.tile([128, ncol], f32, tag="b")
                    for k in range(KC):
                        nc.sync.dma_start(xt[:, k, :], xdr[k * 128:(k + 1) * 128, :])
                        nc.sync.dma_start(wt[:, k, :], wdr[k * 128:(k + 1) * 128, :])
                    nc.sync.dma_start(bt[:], bdr[:])
                    for rt in range((ND + 127) // 128):
                        m = min(128, ND - rt * 128)
                        ps = pp.tile([128, 152], f32, tag="ps")
                        nc.scalar.activation(ps[0:m, 0:ncol], bt[0:m, :], ACTF.Copy)
                        for k in range(KC):
                            nc.tensor.matmul(
                                ps[0:m, 0:ncol],
                                xt[:, k, rt * 128:rt * 128 + m],
                                wt[:, k, :],
                                start=False,
                                stop=(k == KC - 1),
                                skip_group_check=True,
                            )
                        row = rp.tile([128, 256], f16, tag="row")
                        nc.vector.tensor_copy(row[0:m, 0:128], ps[0:m, 0:128])
                        nc.scalar.activation(
                            row[0:m, 128:136], ps[0:m, 128:136], ACTF.Exp)
                        nc.scalar.activation(
                            row[0:m, 136:144], ps[0:m, 128:136], ACTF.Exp, scale=0.2)
                        nc.gpsimd.dma_start(
                            shdr[rt * 128:rt * 128 + m, :], row[0:m, :])
                        if is_news:
                            ed = rp.tile([128, 128], f16, tag="edefrow")
                            nc.scalar.activation(
                                ed[0:m, 0:8], ps[0:m, 136:144], ACTF.Exp)
                            nc.scalar.activation(
                                ed[0:m, 8:16], ps[0:m, 136:144], ACTF.Exp, scale=0.2)
                            nc.scalar.activation(
                                ed[0:m, 16:24], ps[0:m, 144:152], ACTF.Exp)
                            nc.scalar.activation(
                                ed[0:m, 24:32], ps[0:m, 144:152], ACTF.Exp, scale=0.2)
                            nc.gpsimd.dma_start(
                                edefT[rt * 128:rt * 128 + m, 0:32], ed[0:m, 0:32])

                # zero pads: table dummy rows + edefT dummy rows
                zt = rp.tile([128, 256], f16, tag="row")
                nc.vector.memset(zt[:], 0.0)
                nc.gpsimd.dma_start(tabN[NV:NV + 128, :], zt[:])
                nc.gpsimd.dma_start(tabI[NV:NV + 128, :], zt[:])
                nc.gpsimd.dma_start(edefT[ND:NDP, 0:32], zt[0:NDP - ND, 0:32])

            nc.gpsimd.collective_compute(
                "AllGather", mybir.AluOpType.bypass,
                replica_groups=[list(range(NC))],
                ins=[shN[:].opt()], outs=[tabN[0:N_NEWS, :].opt()],
            )
            nc.gpsimd.collective_compute(
                "AllGather", mybir.AluOpType.bypass,
                replica_groups=[list(range(NC))],
                ins=[shI[:].opt()], outs=[tabI[0:N_NEWS, :].opt()],
            )

            # ---------------- message passing ----------------
            with (
                tc.tile_pool(name="midx", bufs=1) as ip,
                tc.tile_pool(name="mg", bufs=2) as gp,
                tc.tile_pool(name="mw", bufs=2) as wp,
            ):
                idx_t = {}
                MAXS = int(os.environ.get("KERNEL_MAXSLOTS", "12"))
                MONLY = os.environ.get("KERNEL_MONLY", "")
                if MP:
                    pix_t = ip.tile([128, NDP // 16], i16, tag="pix")
                    nc.sync.dma_start(pix_t[:], pix_d[:])
                    if MONLY != "idx":
                        for j0 in range(0, NB, 15):
                            nc.gpsimd.dma_gather(
                                edef_all[:, j0:j0 + 15, :], edefT[:],
                                pix_t[:, j0 * 8:(j0 + 15) * 8],
                                128 * 15, 128 * 15, 128)
                    for t, (idr, s) in enumerate(((inn_d, snn), (iin_d, sin))):
                        it = ip.tile([128, 8 * s], i16, tag=f"idx{t}",
                                     name=f"idx{t}")
                        nc.sync.dma_start(it[:], idr[:])
                        idx_t[t] = it
                if MONLY in ("idx", "edef"):
                    idx_t = {}
                for t, (Dv, tabd) in enumerate(((Dnn, tabN), (Din, tabI))):
                    if t not in idx_t:
                        continue
                    c0 = 16 * t
                    off = 0
                    NBLK = int(os.environ.get("KERNEL_NBLK", str(NB)))
                    for b in range((NB if MP else 0)):
                        Db = int(Dv[b])
                        if Db == 0 or b >= NBLK:
                            continue
                        g = gp.tile([128, Db, 256], f16, tag="g")
                        for j0 in range(0, Db, MAXS):
                            jn = min(MAXS, Db - j0)
                            nc.gpsimd.dma_gather(
                                g[:, j0:j0 + jn, :], tabd[:],
                                idx_t[t][:, off + 8 * j0:off + 8 * (j0 + jn)],
                                128 * jn, 128 * jn, 256,
                            )
                        off += 8 * Db
                        if MONLY == "gather":
                            nc.vector.tensor_copy(
                                outR[t][:, b, 0:8], g[:, 0, 0:8])
                            continue
                        edb = edef_all[:, b, :]
                        exa = wp.tile([128, Db, 8], f16, tag="exa")
                        nc.vector.tensor_tensor(
                            exa[:], g[:, :, 128:136],
                            edb[:, c0:c0 + 8].unsqueeze(1).broadcast_to(
                                [128, Db, 8]),
                            ALU.mult)
                        exb = wp.tile([128, Db, 8], f16, tag="exb")
                        nc.vector.tensor_tensor(
                            exb[:], g[:, :, 136:144],
                            edb[:, c0 + 8:c0 + 16].unsqueeze(1).broadcast_to(
                                [128, Db, 8]),
                            ALU.mult)
                        nc.vector.tensor_tensor(exa[:], exa[:], exb[:], ALU.max)
                        # denominator: reduce slots (strided view, innermost=slot)
                        den = wp.tile([128, 8], f32, tag="den")
                        nc.vector.tensor_reduce(
                            den[:].unsqueeze(2),
                            exa[:].transpose([0, 2, 1]),
                            mybir.AxisListType.X, ALU.add)
                        # expand ex across head dims on ACT
                        exe = wp.tile([128, Db, 128], f16, tag="exe")
                        nc.scalar.activation(
                            exe[:].rearrange("p j (h v) -> p j h v", v=16),
                            exa[:].unsqueeze(3).broadcast_to([128, Db, 8, 16]),
                            ACTF.Copy)
                        # messages
                        msg = wp.tile([128, Db, 128], f16, tag="msg")
                        nc.vector.tensor_tensor(
                            msg[:], g[:, :, 0:128], exe[:], ALU.mult)
                        # numerator: pairwise tree over slots
                        cur = Db
                        while cur > 1:
                            hf = cur // 2
                            nc.vector.tensor_tensor(
                                msg[:, 0:hf, :], msg[:, 0:hf, :],
                                msg[:, hf:2 * hf, :], ALU.add)
                            if cur % 2:
                                nc.vector.tensor_tensor(
                                    msg[:, 0:1, :], msg[:, 0:1, :],
                                    msg[:, 2 * hf:cur, :], ALU.add)
                            cur = hf
                        rec = wp.tile([128, 8], f32, tag="rec")
                        nc.vector.tensor_scalar(
                            den[:], den[:], 1e-16, None, ALU.add)
                        nc.vector.reciprocal(rec[:], den[:])
                        rec16 = wp.tile([128, 8], f16, tag="rec16")
                        nc.vector.tensor_copy(rec16[:], rec[:])
                        ob = wp.tile([128, 128], f16, tag="ob")
                        nc.vector.tensor_tensor(
                            ob[:].rearrange("p (h v) -> p h v", v=16),
                            msg[:, 0, :].rearrange("p (h v) -> p h v", v=16),
                            rec16[:].unsqueeze(2).broadcast_to([128, 8, 16]),
                            ALU.mult)
                        nc.vector.tensor_scalar(
                            outR[t][:, b, :], ob[:], 0.0, None, ALU.max)

            # ---------------- semantic attention scores ----------------
            with (
                tc.tile_pool(name="sem", bufs=4) as sp,
                tc.tile_pool(name="sps", bufs=2, space="PSUM") as qp,
            ):
                scr = sp.tile([128, 128], f16, tag="scr")
                for t in range(2):
                    nc.vector.memset(sacc[t][:], 0.0)
                    for b in range(NB if SEM else 0):
                        tp = qp.tile([128, 128], f16, tag="tp")
                        nc.tensor.transpose(tp[:], outR[t][:, b, :], ident[:])
                        tps = sp.tile([128, 128], f16, tag="tps")
                        nc.scalar.activation(tps[:], tp[:], ACTF.Copy)
                        ps2 = qp.tile([128, 128], f32, tag="ps2")
                        nc.scalar.activation(ps2[:], bk_t[:], ACTF.Copy)
                        nc.tensor.matmul(
                            ps2[:], tps[:], wk_t[:],
                            start=False, stop=True, skip_group_check=True)
                        th = sp.tile([128, 128], f16, tag="th")
                        nc.scalar.activation(th[:], ps2[:], ACTF.Tanh)
                        qacc = sp.tile([128, 1], f32, tag="qacc")
                        nc.vector.tensor_tensor(
                            scr[:], th[:], q_t[:], ALU.mult)
                        nc.vector.tensor_reduce(
                            qacc[:].unsqueeze(2), scr[:].unsqueeze(1),
                            mybir.AxisListType.X, ALU.add)
                        nc.vector.tensor_tensor(
                            sacc[t][:], sacc[t][:], qacc[:], ALU.add)

                spart = sp.tile([128, 2], f32, tag="spart")
                if SEM:
                    pr0 = sp.tile([128, 1], f32, tag="pr0")
                    pr1 = sp.tile([128, 1], f32, tag="pr1")
                    nc.gpsimd.partition_all_reduce(
                        pr0[:], sacc[0][:], 128, bass_isa.ReduceOp.add)
                    nc.gpsimd.partition_all_reduce(
                        pr1[:], sacc[1][:], 128, bass_isa.ReduceOp.add)
                    nc.vector.tensor_copy(spart[:, 0:1], pr0[:])
                    nc.vector.tensor_copy(spart[:, 1:2], pr1[:])
                else:
                    nc.vector.memset(spart[:], 0.0)
                nc.gpsimd.dma_start(arI[:], spart[:])
                nc.gpsimd.collective_compute(
                    "AllReduce", mybir.AluOpType.add,
                    replica_groups=[list(range(NC))],
                    ins=[arI[:].opt()], outs=[arO[:].opt()],
                )
                sc = sp.tile([128, 2], f32, tag="sc")
                nc.sync.dma_start(sc[:], arO[:])
                nc.vector.tensor_scalar(
                    sc[:], sc[:], 1.0 / N_NEWS, None, ALU.mult)
                eb = sp.tile([128, 2], f32, tag="eb")
                nc.scalar.activation(eb[:], sc[:], ACTF.Exp)
                ebs = sp.tile([128, 1], f32, tag="ebs")
                nc.vector.tensor_tensor(
                    ebs[:], eb[:, 0:1], eb[:, 1:2], ALU.add)
                ebr = sp.tile([128, 1], f32, tag="ebr")
                nc.vector.reciprocal(ebr[:], ebs[:])
                beta = sp.tile([128, 2], f32, tag="beta")
                nc.vector.tensor_scalar(
                    beta[:], eb[:], ebr[:], None, ALU.mult)

                # ---------------- fuse + elu + head ----------------
                for b in range(NB):
                    fa = sp.tile([128, 128], f16, tag="fa")
                    nc.vector.tensor_scalar(
                        fa[:], outR[0][:, b, :], beta[:, 0:1], None, ALU.mult)
                    fb = sp.tile([128, 128], f16, tag="fb")
                    nc.vector.tensor_scalar(
                        fb[:], outR[1][:, b, :], beta[:, 1:2], None, ALU.mult)
                    nc.vector.tensor_tensor(fa[:], fa[:], fb[:], ALU.add)
                    ng = sp.tile([128, 128], f16, tag="ng")
                    nc.vector.tensor_scalar(
                        ng[:], fa[:], 0.0, None, ALU.min)
                    en = sp.tile([128, 128], f16, tag="en")
                    nc.scalar.activation(en[:], ng[:], ACTF.Exp)
                    nc.vector.tensor_scalar(
                        fa[:], fa[:], 0.0, None, ALU.max)
                    nc.vector.tensor_tensor(fa[:], fa[:], en[:], ALU.add)
                    nc.vector.tensor_scalar(
                        fa[:], fa[:], 1.0, None, ALU.subtract)
                    tp2 = qp.tile([128, 128], f16, tag="tp")
                    nc.tensor.transpose(tp2[:], fa[:], ident[:])
                    tps2 = sp.tile([128, 128], f16, tag="tps")
                    nc.scalar.activation(tps2[:], tp2[:], ACTF.Copy)
                    ps4 = qp.tile([128, C_OUT], f32, tag="ps4")
                    nc.scalar.activation(ps4[:], bo_t[:], ACTF.Copy)
                    nc.tensor.matmul(
                        ps4[:], tps2[:], wo_t[:],
                        start=False, stop=True, skip_group_check=True)
                    o4 = sp.tile([128, C_OUT], f32, tag="o4")
                    nc.vector.tensor_copy(o4[:], ps4[:])
                    nc.gpsimd.dma_start(
                        out_d[b * 128:(b + 1) * 128, :], o4[:])

    nc.compile()
    return nc


# --------------------------------------------------------------------------
# kernel entry
# --------------------------------------------------------------------------
def kernel(**inputs) -> np.ndarray:
    inp = {k: np.asarray(v) for k, v in inputs.items()}
    e_in_src = np.asarray(inp["edge_in"][0])
    if e_in_src.size and int(e_in_src.max()) >= N_NEWS:
        sys.stderr.write("[kernel] edge_in src >= 30000; host fallback\n")
        return _np_full_reference(inp)

    f32 = np.float32
    Wn, Wi = inp["W_news"].astype(f32), inp["W_inter"].astype(f32)
    bn, bi = inp["b_news"].astype(f32), inp["b_inter"].astype(f32)
    Asn, Adn = _a_fold(inp["a_src_nn"].astype(f32)), _a_fold(inp["a_dst_nn"].astype(f32))
    Asi, Adi = _a_fold(inp["a_src_in"].astype(f32)), _a_fold(inp["a_dst_in"].astype(f32))
    # fused projections: news [W | W@Asn | W@Adn | W@Adi], inter [W | W@Asi]
    Wn_f = np.concatenate([Wn, Wn @ Asn, Wn @ Adn, Wn @ Adi], 1).astype(np.float16)
    Wi_f = np.concatenate([Wi, Wi @ Asi], 1).astype(np.float16)
    bn_f = np.concatenate([bn, bn @ Asn, bn @ Adn, bn @ Adi]).astype(f32)
    bi_f = np.concatenate([bi, bi @ Asi]).astype(f32)
    bn_rep = np.tile(bn_f[None, :], (128, 1))
    bi_rep = np.tile(bi_f[None, :], (128, 1))

    # ---- per-core edge preprocessing (integer only) ----
    mats_nn, mats_in, degs_nn, degs_in, perms = [], [], [], [], []
    for c in range(NC):
        lo = c * ND
        mn, dn = _prep_edges(inp["edge_nn"], lo)
        mi, di = _prep_edges(inp["edge_in"], lo)
        perms.append(np.argsort(dn + di, kind="stable"))
        mats_nn.append(mn); mats_in.append(mi)
        degs_nn.append(dn); degs_in.append(di)

    # dummies go in front (cheapest block), then ascending real dsts
    full_perms = [np.concatenate([np.arange(ND, NDP, dtype=np.int64), perms[c]])
                  for c in range(NC)]

    # global per-block slot counts (uniform across cores)
    Dnn = np.zeros(NB, np.int64)
    Din = np.zeros(NB, np.int64)
    for c in range(NC):
        fp = full_perms[c]
        dn = np.concatenate([degs_nn[c], np.zeros(NDP - ND, np.int64)])
        di = np.concatenate([degs_in[c], np.zeros(NDP - ND, np.int64)])
        for b in range(NB):
            rows = fp[b * 128:(b + 1) * 128]
            Dnn[b] = max(Dnn[b], dn[rows].max())
            Din[b] = max(Din[b], di[rows].max())

    def build_idx(mat, Dv, fp):
        segs = []
        nd_rows = mat.shape[0]
        for b in range(NB):
            Db = int(Dv[b])
            if Db == 0:
                continue
            rows = fp[b * 128:(b + 1) * 128]
            blk = np.full((128, Db), NV, np.int64)
            real = rows < nd_rows
            take = min(Db, mat.shape[1])
            blk[real, :take] = mat[rows[real], :take]
            segs.append(blk.T.reshape(-1))  # slot-major: j = c*128 + p
        return _wrap_idx(np.concatenate(segs)) if segs else np.zeros(
            (128, 0), np.int16)

    in_maps = []
    xn16 = inp["x_news"].astype(np.float16)
    xi16 = inp["x_inter"][:N_NEWS].astype(np.float16)
    wk16 = inp["Wk"].astype(np.float16)
    bk_rep = np.tile(inp["bk"].astype(f32)[None, :], (128, 1))
    q_rep = np.tile(inp["q"].astype(np.float16)[None, :], (128, 1))
    wo16 = inp["W_out"].astype(np.float16)
    bo_rep = np.tile(inp["b_out"].astype(f32)[None, :], (128, 1))
    ident = np.eye(128, dtype=np.float16)
    for c in range(NC):
        lo = c * ND
        fp = full_perms[c]
        in_maps.append({
            "xnT": np.ascontiguousarray(xn16[lo:lo + ND].T),
            "xiT": np.ascontiguousarray(xi16[lo:lo + ND].T),
            "wn": Wn_f, "wi": Wi_f, "bn": bn_rep, "bi": bi_rep,
            "idx_nn": build_idx(mats_nn[c], Dnn, fp),
            "idx_in": build_idx(mats_in[c], Din, fp),
            "pidx": _wrap_idx(fp),
            "wk": wk16, "bkr": bk_rep, "qr": q_rep,
            "wo": wo16, "bor": bo_rep, "ident": ident,
        })

    use_device = os.environ.get("KERNEL_NO_DEVICE", "0") != "1"
    if use_device:
        try:
            from concourse.bass_utils import run_bass_kernel_spmd

            nc = _build_graph(Dnn, Din)
            t0 = time.time()
            res = run_bass_kernel_spmd(nc, in_maps, list(range(NC)))
            _LAST["ns"] = int((time.time() - t0) * 1e9)
            _LAST["nc"] = nc
            _LAST["in_maps"] = in_maps
            out = np.zeros((N_NEWS, C_OUT), f32)
            for c in range(NC):
                o = res.results[c]["out4"]
                fp = full_perms[c]
                real = fp < ND
                out[c * ND + fp[real]] = o[real]
            return out
        except Exception as exc:  # pragma: no cover
            import traceback
            traceback.print_exc()
            sys.stderr.write(f"[kernel] device failed ({exc!r}); host fallback\n")
    return _np_full_reference(inp)
